# revision 46
# baseline (speedup 1.0000x reference)
"""Trainium2 Bass kernel for nn_GAT_22462678958399 (dense-GAT + MLP head).

Data-parallel over the 4096-graph batch across 8 NeuronCores (512/core).
The attention pipeline runs entirely in the MLP's transposed [k, g] layout
(k = i*116 + j padded to 106 chunks of 128), so the baseline's [g, k]
elementwise passes and 128x128 PE transposes disappear:

  z_c   = S1_c @ f1T + S2_c @ f2T    one dual-fp8 DoubleRow matmul per
                                     chunk (packed 0/1 selectors; F = x @
                                     W_att @ [a1,a2] is folded on host)
  zl    = Prelu(z, 0.2)              ACT (Prelu shares the Exp act table)
  p     = Exp(zl)                    ACT, fp16
  pm    = p * adjT(0/1)              DVE / Pool (alternating)
  s    += (R_c/32) @ pm              PE, f32 psum accumulation
  rS    = 1/s (= 32/s)               DVE reciprocal -> fp8
  rf_c  = S1_c @ rS                  PE (fp8)
  att_c = pm * rf -> fp8e4 (att*32)  DVE
  hp   += W1_c^T @ att (DoubleRow)   PE, dual-fp8 (W1 host-scaled x16)
  h1    = Relu(hp + 512*b1) fp16; h2 = w2^T @ h1; log_softmax tail.

Emission is software-pipelined for the in-order engines: stage B of half h
is interleaved with stage A of half h+1, the softmax-sum matmuls trail
their producers by SLAG pairs and the MLP matmuls trail the normalize by
BLAG pairs, so PE never blocks on ACT/DVE latency.

Quantization (validated vs f64 in CoreSim and on HW): output absmax err
~1.9e-3 on a 0.72-absmax output (gate 1.45e-2).
"""

import numpy as np
import ml_dtypes

import concourse.bass as bass
import concourse.bacc as bacc
import concourse.mybir as mybir
import concourse.tile as tile
from concourse.bass_utils import run_bass_kernel_spmd

F8NP = ml_dtypes.float8_e4m3  # IEEE e4m3 (max 240) == TRN fp8e4

N = 116
NN = N * N
NKC = 106
KPAD = NKC * 128
ND = NKC // 2
B = 4096
NCORES = 8
G = 512
GH = 256
ATT_SCALE = 32.0
W1_SCALE = 16.0
HSCALE = ATT_SCALE * W1_SCALE
ASPLIT = (18, 18, 17)
SLAG = 10
BLAG = 8
# Prelu per a-step: ACT 1-pass (Prelu) for a fraction of steps; else DVE
# 2-pass (t=0.2z ts, max(t,z) tt — one PSUM read each; gpsimd can't touch
# PSUM so both passes sit on DVE).
# HW-calibrated (micro.py): ACT act 490ns, DVE fp16 SBUF tt 249ns, DVE
# PSUM ts/tt ~290-330ns, Pool tt 915ns (useless for elementwise), PE
# [*,256] matmul ~73-80ns.  Phase 2+3 are PE-bound; phase 1 is ACT/DVE.
AF_PHASE1 = 0.344
AF_PHASE2 = 0.85
# w1 DMA queue per load in the tail B pass (no A work to compete with)
W1_TAIL_ENGS = ("sync", "scalar", "gpsimd")

f32 = mybir.dt.float32
fp16 = mybir.dt.float16
fp8 = mybir.dt.float8e4

AL = mybir.AluOpType
AF = mybir.ActivationFunctionType
PM = mybir.MatmulPerfMode


def build_nc(reps=1):
    nc = bacc.Bacc("TRN2", target_bir_lowering=False, debug=False)

    F12 = nc.dram_tensor("f12", [N, 1024], fp8, kind="ExternalInput")
    MADJ = nc.dram_tensor("madjt", [2, 128, NKC * GH], fp16, kind="ExternalInput")
    W1 = nc.dram_tensor("w1q", [27, 128, 4096], fp8, kind="ExternalInput")
    S12 = nc.dram_tensor("s12sel", [N, NKC * 256], fp8, kind="ExternalInput")
    RSEL = nc.dram_tensor("rsel", [128, NKC * N], fp16, kind="ExternalInput")
    W2 = nc.dram_tensor("w2r", [128, 16], fp16, kind="ExternalInput")
    B1 = nc.dram_tensor("b1r", [128, 8], f32, kind="ExternalInput")
    B2 = nc.dram_tensor("b2c", [1, 2], f32, kind="ExternalInput")
    OUT = nc.dram_tensor("out", [2, G], f32, kind="ExternalOutput")

    from contextlib import ExitStack
    with tile.TileContext(nc) as tc:
        with ExitStack() as es:
            pool = lambda name, bufs, space="SBUF": es.enter_context(
                tc.tile_pool(name=name, bufs=bufs, space=space))
            cpool = pool("const", 1)
            mpool = pool("madj", 3)
            tpool2 = pool("t02", 6)
            zlpool = pool("zl", 8)
            pmpool = pool("pm", ND)
            apool = pool("att", 4)
            wpool = pool("w1", 6)
            hpool = pool("h1", 1)
            tpool = pool("tail", 1)
            rspool = pool("rs", 2)
            # 8 PSUM banks total: psZR(3) + psB(4) + psA(1).  psB doubles as
            # extra z-depth for A(0) in phase 1 (the MLP accumulators are
            # idle then) and as the hp accumulators from phase 2 on — the
            # pool's WAR tracking serializes the handoff.
            psZR = pool("psZR", 3, "PSUM")
            psB = pool("psB", 4, "PSUM")
            psA = pool("psA", 1, "PSUM")

            f12s = cpool.tile_from(F12[:])
            # chunked constant loads: early chunks unblock the first z
            # matmuls ~10us sooner than one monolithic DMA would
            s12s = cpool.tile([N, NKC * 256], fp8, tag="s12s", name="s12s")
            for i in range(4):
                lo = i * 27 * 256
                hi = min(NKC, (i + 1) * 27) * 256
                nc.sync.dma_start(s12s[:, lo:hi], S12[:, lo:hi])
            rsels = cpool.tile([128, NKC * N], fp16, tag="rsels",
                               name="rsels")
            for i in range(2):
                lo = i * 53 * N
                hi = min(NKC, (i + 1) * 53) * N
                nc.sync.dma_start(rsels[:, lo:hi], RSEL[:, lo:hi])
            w2s = cpool.tile_from(W2[:])
            b1s = cpool.tile_from(B1[:])
            b2s = cpool.tile_from(B2[:])

            zb128 = cpool.tile([128, 1], f32, tag="zb128", name="zb128")
            nc.vector.memset(zb128[:], 0.0)
            zb1 = cpool.tile([1, 1], f32, tag="zb1", name="zb1")
            nc.vector.memset(zb1[:], 0.0)
            ca = tpool.tile([1, G], f32, tag="ca", name="ca")
            cb = tpool.tile([1, G], f32, tag="cb", name="cb")
            ta = tpool.tile([1, G], f32, tag="ta", name="ta")
            tb = tpool.tile([1, G], f32, tag="tb", name="tb")

            abnd = []
            o = 0
            for nt in ASPLIT:
                abnd.append((o, o + nt))
                o += nt

            def emit_body():
              class HState:
                  pass

              def a_init(h, act_frac, zpools):
                  st = HState()
                  st.h = h
                  st.sP = psA.tile([N, GH], f32, tag="aux", name=f"s{h}")
                  st.pms = []
                  st.pend = []
                  st.mslab = None
                  st.act_frac = act_frac
                  st.acc = 0.0
                  st.zpools = zpools
                  return st

              def emit_sum(st, c, pmv):
                  nc.tensor.matmul(
                      st.sP[:], rsels[:, c * N:(c + 1) * N], pmv,
                      start=(c == 0), stop=(c == NKC - 1))

              def a_step(st, d):
                  h = st.h
                  c0, c1 = 2 * d, 2 * d + 1
                  if d % 4 == 0:
                      mw = min(2048, NKC * GH - d * 512)
                      st.mslab = mpool.tile([128, mw], fp16, tag="madj")
                      nc.sync.dma_start(
                          st.mslab[:], MADJ[h][:, d * 512:d * 512 + mw])
                  zp = st.zpools[d % len(st.zpools)]
                  z = zp.tile([128, 512], f32,
                              tag=("hp" if zp is psB else "zr"))
                  f12h = (f12s[:, h * 512:(h + 1) * 512]
                          .rearrange("p (two g) -> p two g", two=2))
                  for j, c in ((0, c0), (1, c1)):
                      s12v = (s12s[:, c * 256:(c + 1) * 256]
                              .rearrange("p (two m) -> p two m", two=2))
                      nc.tensor.matmul(
                          z[:, j * GH:(j + 1) * GH], s12v, f12h,
                          perf_mode=PM.DoubleRow,
                          start=(j == 0), stop=(j == 1),
                          skip_group_check=True)
                  zl = zlpool.tile([128, 512], fp16, tag="zl")
                  st.acc += st.act_frac
                  if st.acc >= 1.0:
                      st.acc -= 1.0
                      nc.scalar.activation(zl[:], z[:], AF.Prelu,
                                           bias=zb128[:, 0:1], alpha=0.2)
                  else:
                      t02 = tpool2.tile([128, 512], fp16, tag="t02")
                      nc.vector.tensor_scalar(t02[:], z[:], 0.2, None,
                                              op0=AL.mult)
                      nc.vector.tensor_tensor(zl[:], t02[:], z[:], op=AL.max)
                  pm = pmpool.tile([128, 512], fp16, tag="pm")
                  nc.scalar.activation(pm[:], zl[:], AF.Exp,
                                       bias=zb128[:, 0:1])
                  nc.vector.tensor_tensor(
                      pm[:], pm[:],
                      st.mslab[:, (d % 4) * 512:(d % 4) * 512 + 512],
                      op=AL.mult)
                  st.pms.append(pm)
                  st.pend.append((c0, pm[:, 0:GH]))
                  st.pend.append((c1, pm[:, GH:512]))
                  while len(st.pend) > 2 * SLAG:
                      emit_sum(st, *st.pend.pop(0))

              def a_finish(st):
                  for args in st.pend:
                      emit_sum(st, *args)
                  st.pend = []
                  rS = rspool.tile([N, GH], fp8, tag="rs")
                  with nc.allow_low_precision(reason="32/s fp8 validated"):
                      nc.vector.reciprocal(rS[:], st.sP[:])
                  st.rS = rS

              def b_init(st, tail=False):
                  st.hps = [psB.tile([128, 512], f32, tag="hp",
                                     name=f"hp{st.h}_{q}") for q in range(4)]
                  st.ati = -1
                  st.att = None
                  st.avs = {}
                  st.w1vs = {}
                  st.bpend = []
                  st.tail = tail

              def emit_mlp(st, d):
                  attv = st.avs[d].rearrange("p (two g) -> p two g", two=2)
                  w1v = st.w1vs[d].rearrange("p (two oc) -> p two oc", two=2)
                  for oc in range(8):
                      nc.tensor.matmul(
                          st.hps[oc // 2][:, (oc % 2) * GH:(oc % 2 + 1) * GH],
                          w1v[:, :, oc * 128:(oc + 1) * 128], attv,
                          start=(d == 0 and oc % 2 == 0),
                          stop=(d == ND - 1),
                          perf_mode=PM.DoubleRow, skip_group_check=True)

              def b_step(st, d):
                  c0, c1 = 2 * d, 2 * d + 1
                  if st.att is None or d >= abnd[st.ati][1]:
                      st.ati += 1
                      st.att = apool.tile(
                          [128, (abnd[st.ati][1] - abnd[st.ati][0]) * 512],
                          fp8, tag="att")
                  a0 = abnd[st.ati][0]
                  if d % 2 == 0:
                      w1t = wpool.tile([128, 4096], fp8, tag="w1")
                      weng = (getattr(nc, W1_TAIL_ENGS[(d // 2) % len(W1_TAIL_ENGS)])
                              if st.tail else nc.gpsimd)
                      weng.dma_start(w1t[:], W1[d // 2])
                      st.w1vs[d] = w1t[:, 0:2048]
                      if d + 1 < ND:
                          st.w1vs[d + 1] = w1t[:, 2048:4096]
                  rf = psZR.tile([128, 512], f32, tag="zr")
                  nc.tensor.matmul(
                      rf[:, 0:GH], s12s[:, c0 * 256:c0 * 256 + 128],
                      st.rS[:], start=True, stop=False, skip_group_check=True)
                  nc.tensor.matmul(
                      rf[:, GH:512], s12s[:, c1 * 256:c1 * 256 + 128],
                      st.rS[:], start=False, stop=True, skip_group_check=True)
                  av = st.att[:, (d - a0) * 512:(d - a0 + 1) * 512]
                  nc.vector.tensor_tensor(av, st.pms[d][:], rf[:],
                                          op=AL.mult)
                  st.avs[d] = av
                  st.bpend.append(d)
                  while len(st.bpend) > BLAG:
                      emit_mlp(st, st.bpend.pop(0))

              def b_finish(st):
                  h = st.h
                  for d in st.bpend:
                      emit_mlp(st, d)
                  st.bpend = []
                  h1 = hpool.tile([128, 8 * GH], fp16, tag="h1")
                  for oc in range(8):
                      nc.scalar.activation(
                          h1[:, oc * GH:(oc + 1) * GH],
                          st.hps[oc // 2][:, (oc % 2) * GH:(oc % 2 + 1) * GH],
                          AF.Relu, bias=b1s[:, oc:oc + 1], scale=1.0)
                  h2t = psZR.tile([128, 512], f32, tag="zr")
                  h2 = h2t[0:1, :]
                  for oc in range(8):
                      for cls in range(2):
                          nc.tensor.matmul(
                              h2[0:1, cls * GH:(cls + 1) * GH],
                              w2s[:, 2 * oc + cls:2 * oc + cls + 1],
                              h1[:, oc * GH:(oc + 1) * GH],
                              start=(oc == 0 and cls == 0), stop=(oc == 7),
                              skip_group_check=True)
                  nc.scalar.activation(ca[:, h * GH:(h + 1) * GH],
                                       h2[0:1, 0:GH], AF.Identity,
                                       bias=b2s[0:1, 0:1], scale=1.0 / HSCALE)
                  nc.scalar.activation(cb[:, h * GH:(h + 1) * GH],
                                       h2[0:1, GH:2 * GH], AF.Identity,
                                       bias=b2s[0:1, 1:2], scale=1.0 / HSCALE)

              # ---- pipeline: A(0) | B(0)+A(1) | B(1) ----
              st0 = a_init(0, AF_PHASE1, [psZR, psB])
              for d in range(ND):
                  a_step(st0, d)
              a_finish(st0)

              st1 = a_init(1, AF_PHASE2, [psZR])
              b_init(st0)
              for d in range(ND):
                  b_step(st0, d)
                  a_step(st1, d)
              b_finish(st0)
              a_finish(st1)

              b_init(st1, tail=True)
              for d in range(ND):
                  b_step(st1, d)
              b_finish(st1)

              # ---- log_softmax tail ----
              nc.vector.tensor_tensor(ta[:], ca[:], cb[:], op=AL.max)
              nc.vector.tensor_tensor(ca[:], ca[:], ta[:], op=AL.subtract)
              nc.vector.tensor_tensor(cb[:], cb[:], ta[:], op=AL.subtract)
              nc.scalar.activation(ta[:], ca[:], AF.Exp, bias=zb1[:, 0:1])
              nc.scalar.activation(tb[:], cb[:], AF.Exp, bias=zb1[:, 0:1])
              nc.vector.tensor_tensor(ta[:], ta[:], tb[:], op=AL.add)
              nc.scalar.activation(ta[:], ta[:], AF.Ln, bias=zb1[:, 0:1])
              nc.vector.tensor_tensor(ca[:], ca[:], ta[:], op=AL.subtract)
              nc.vector.tensor_tensor(cb[:], cb[:], ta[:], op=AL.subtract)
              nc.sync.dma_start(OUT[0:1, :], ca[:])
              nc.sync.dma_start(OUT[1:2, :], cb[:])

            if reps == 1:
                emit_body()
            else:
                with tc.For_i(0, reps, 1) as _i:
                    emit_body()

    return nc


def _sel_arrays():
    k = np.arange(KPAD)
    ik = k // N
    jk = k % N
    valid = k < NN
    s12 = np.zeros((N, NKC * 256), F8NP)
    kk = k[valid]
    s12[ik[valid], (kk // 128) * 256 + kk % 128] = 1.0
    s12[jk, (k // 128) * 256 + 128 + k % 128] = 1.0
    rsel = np.zeros((128, NKC * N), np.float16)
    rsel[kk % 128, (kk // 128) * N + ik[valid]] = 1.0 / ATT_SCALE
    return s12, rsel


def _prep_inputs(x, adj, W_att, a1, a2, W1, b1, W2, b2):
    x = np.asarray(x, np.float32)
    adj = np.asarray(adj, np.float32)
    v12 = (np.asarray(W_att, np.float64)
           @ np.stack([np.asarray(a1, np.float64),
                       np.asarray(a2, np.float64)], 1))
    F = (x.reshape(B * N, N).astype(np.float64) @ v12).astype(np.float32)
    F = F.reshape(NCORES, G, N, 2)
    f12 = np.empty((NCORES, N, 2, 2, GH), np.float32)
    for hh in range(2):
        gs = slice(hh * GH, (hh + 1) * GH)
        f12[:, :, hh, 0, :] = F[:, gs, :, 0].transpose(0, 2, 1)
        f12[:, :, hh, 1, :] = F[:, gs, :, 1].transpose(0, 2, 1)
    f12 = np.clip(f12, -240, 240).astype(F8NP).reshape(NCORES, N, 1024)

    m01 = (adj > 0).astype(np.float16).reshape(NCORES, 2, GH, NN)
    mpad = np.zeros((NCORES, 2, GH, KPAD), np.float16)
    mpad[..., :NN] = m01
    madjt = np.ascontiguousarray(
        mpad.reshape(NCORES, 2, GH, NKC, 128).transpose(0, 1, 4, 3, 2)
        .reshape(NCORES, 2, 128, NKC * GH))

    W1p = np.zeros((KPAD, 1024), np.float32)
    W1p[:NN] = np.asarray(W1, np.float32) * W1_SCALE
    w1q = np.clip(W1p, -240, 240).astype(F8NP)
    w1q = np.ascontiguousarray(
        w1q.reshape(ND, 2, 128, 1024).transpose(0, 2, 1, 3)
        .reshape(ND, 128, 2048))
    w1p2 = np.zeros((54, 128, 2048), F8NP)
    w1p2[:ND] = w1q
    w1q = np.ascontiguousarray(
        w1p2.reshape(27, 2, 128, 2048).transpose(0, 2, 1, 3)
        .reshape(27, 128, 4096))

    w2r = np.ascontiguousarray(
        np.asarray(W2, np.float32).reshape(8, 128, 2)
        .transpose(1, 0, 2).reshape(128, 16)).astype(np.float16)
    b1r = np.ascontiguousarray(
        (np.asarray(b1, np.float32) * HSCALE).reshape(8, 128).T)
    b2c = np.ascontiguousarray(np.asarray(b2, np.float32).reshape(1, 2))

    s12, rsel = _sel_arrays()
    consts = dict(w1q=w1q, s12sel=s12, rsel=rsel,
                  w2r=w2r, b1r=b1r, b2c=b2c)
    return [dict(consts, f12=f12[c], madjt=madjt[c])
            for c in range(NCORES)]


TRACE = False
LAST_RESULTS = None


def kernel(x, adj, W_att, a1, a2, W1, b1, W2, b2):
    global LAST_RESULTS
    in_maps = _prep_inputs(x, adj, W_att, a1, a2, W1, b1, W2, b2)
    nc = build_nc()
    nc.compile()
    bres = run_bass_kernel_spmd(nc, in_maps, list(range(NCORES)), trace=TRACE)
    LAST_RESULTS = bres
    res = bres.results
    out = np.empty((B, 2), np.float32)
    for c in range(NCORES):
        out[c * G:(c + 1) * G] = np.asarray(res[c]["out"]).T
    return out



# revision 47
# speedup vs baseline: 1.1495x; 1.1495x over previous
"""Trainium2 Bass kernel for nn_GAT_22462678958399 (dense-GAT + MLP head).

Data-parallel over the 4096-graph batch across 8 NeuronCores (512/core).
The attention pipeline runs entirely in the MLP's transposed [k, g] layout
(k = i*116 + j padded to 106 chunks of 128), so the baseline's [g, k]
elementwise passes and 128x128 PE transposes disappear:

  z_c   = S1_c @ f1T + S2_c @ f2T    one dual-fp8 DoubleRow matmul per
                                     chunk (packed 0/1 selectors; F = x @
                                     W_att @ [a1,a2] is folded on host)
  zl    = Prelu(z, 0.2)              ACT (Prelu shares the Exp act table)
  p     = Exp(zl)                    ACT, fp16
  pm    = p * adjT(0/1)              DVE / Pool (alternating)
  s    += (R_c/32) @ pm              PE, f32 psum accumulation
  rS    = 1/s (= 32/s)               DVE reciprocal -> fp8
  rf_c  = S1_c @ rS                  PE (fp8)
  att_c = pm * rf -> fp8e4 (att*32)  DVE
  hp   += W1_c^T @ att (DoubleRow)   PE, dual-fp8 (W1 host-scaled x16)
  h1    = Relu(hp + 512*b1) fp16; h2 = w2^T @ h1; log_softmax tail.

Emission is software-pipelined for the in-order engines: stage B of half h
is interleaved with stage A of half h+1, the softmax-sum matmuls trail
their producers by SLAG pairs and the MLP matmuls trail the normalize by
BLAG pairs, so PE never blocks on ACT/DVE latency.

Quantization (validated vs f64 in CoreSim and on HW): output absmax err
~1.9e-3 on a 0.72-absmax output (gate 1.45e-2).
"""

import numpy as np
import ml_dtypes

import concourse.bass as bass
import concourse.bacc as bacc
import concourse.mybir as mybir
import concourse.tile as tile
from concourse.bass_utils import run_bass_kernel_spmd

F8NP = ml_dtypes.float8_e4m3  # IEEE e4m3 (max 240) == TRN fp8e4

N = 116
NN = N * N
NKC = 106
KPAD = NKC * 128
ND = NKC // 2
B = 4096
NCORES = 8
G = 512
GH = 256
ATT_SCALE = 32.0
W1_SCALE = 16.0
HSCALE = ATT_SCALE * W1_SCALE
ASPLIT = (18, 18, 17)
SLAG = 10
BLAG = 8
# Prelu per a-step: ACT 1-pass (Prelu) for a fraction of steps; else DVE
# 2-pass (t=0.2z ts, max(t,z) tt — one PSUM read each; gpsimd can't touch
# PSUM so both passes sit on DVE).
# HW-calibrated (micro.py): ACT act 490ns, DVE fp16 SBUF tt 249ns, DVE
# PSUM ts/tt ~290-330ns, Pool tt 915ns (useless for elementwise), PE
# [*,256] matmul ~73-80ns.  Phase 2+3 are PE-bound; phase 1 is ACT/DVE.
AF_PHASE1 = 0.344
AF_PHASE2 = 1.0
# w1 DMA queue per load in the tail B pass (no A work to compete with)
W1_TAIL_ENGS = ("sync", "scalar", "gpsimd")

f32 = mybir.dt.float32
fp16 = mybir.dt.float16
fp8 = mybir.dt.float8e4

AL = mybir.AluOpType
AF = mybir.ActivationFunctionType
PM = mybir.MatmulPerfMode


def build_nc(reps=1):
    nc = bacc.Bacc("TRN2", target_bir_lowering=False, debug=False)

    F12 = nc.dram_tensor("f12", [N, 1024], fp8, kind="ExternalInput")
    MADJ = nc.dram_tensor("madjt", [2, 128, NKC * GH], fp16, kind="ExternalInput")
    W1 = nc.dram_tensor("w1q", [27, 128, 4096], fp8, kind="ExternalInput")
    S12 = nc.dram_tensor("s12sel", [N, NKC * 256], fp8, kind="ExternalInput")
    RSEL = nc.dram_tensor("rsel", [128, NKC * N], fp16, kind="ExternalInput")
    W2 = nc.dram_tensor("w2r", [128, 16], fp16, kind="ExternalInput")
    B1 = nc.dram_tensor("b1r", [128, 8], f32, kind="ExternalInput")
    B2 = nc.dram_tensor("b2c", [1, 2], f32, kind="ExternalInput")
    OUT = nc.dram_tensor("out", [2, G], f32, kind="ExternalOutput")

    from contextlib import ExitStack
    with tile.TileContext(nc) as tc:
        with ExitStack() as es:
            pool = lambda name, bufs, space="SBUF": es.enter_context(
                tc.tile_pool(name=name, bufs=bufs, space=space))
            cpool = pool("const", 1)
            mpool = pool("madj", 3)
            tpool2 = pool("t02", 6)
            zlpool = pool("zl", 8)
            pmpool = pool("pm", ND)
            apool = pool("att", 4)
            wpool = pool("w1", 6)
            hpool = pool("h1", 1)
            tpool = pool("tail", 1)
            rspool = pool("rs", 2)
            # 8 PSUM banks total: psZR(3) + psB(4) + psA(1).  psB doubles as
            # extra z-depth for A(0) in phase 1 (the MLP accumulators are
            # idle then) and as the hp accumulators from phase 2 on — the
            # pool's WAR tracking serializes the handoff.
            psZR = pool("psZR", 3, "PSUM")
            psB = pool("psB", 4, "PSUM")
            psA = pool("psA", 1, "PSUM")

            f12s = cpool.tile_from(F12[:])
            # chunked constant loads: early chunks unblock the first z
            # matmuls ~10us sooner than one monolithic DMA would
            s12s = cpool.tile([N, NKC * 256], fp8, tag="s12s", name="s12s")
            for i in range(4):
                lo = i * 27 * 256
                hi = min(NKC, (i + 1) * 27) * 256
                nc.sync.dma_start(s12s[:, lo:hi], S12[:, lo:hi])
            rsels = cpool.tile([128, NKC * N], fp16, tag="rsels",
                               name="rsels")
            for i in range(2):
                lo = i * 53 * N
                hi = min(NKC, (i + 1) * 53) * N
                nc.sync.dma_start(rsels[:, lo:hi], RSEL[:, lo:hi])
            w2s = cpool.tile_from(W2[:])
            b1s = cpool.tile_from(B1[:])
            b2s = cpool.tile_from(B2[:])

            zb128 = cpool.tile([128, 1], f32, tag="zb128", name="zb128")
            nc.vector.memset(zb128[:], 0.0)
            zb1 = cpool.tile([1, 1], f32, tag="zb1", name="zb1")
            nc.vector.memset(zb1[:], 0.0)
            ca = tpool.tile([1, G], f32, tag="ca", name="ca")
            cb = tpool.tile([1, G], f32, tag="cb", name="cb")
            ta = tpool.tile([1, G], f32, tag="ta", name="ta")
            tb = tpool.tile([1, G], f32, tag="tb", name="tb")

            abnd = []
            o = 0
            for nt in ASPLIT:
                abnd.append((o, o + nt))
                o += nt

            def emit_body():
              class HState:
                  pass

              def a_init(h, act_frac, zpools):
                  st = HState()
                  st.h = h
                  st.sP = psA.tile([N, GH], f32, tag="aux", name=f"s{h}")
                  st.pms = []
                  st.pend = []
                  st.mslab = None
                  st.act_frac = act_frac
                  st.acc = 0.0
                  st.zpools = zpools
                  return st

              def emit_sum(st, c, pmv):
                  nc.tensor.matmul(
                      st.sP[:], rsels[:, c * N:(c + 1) * N], pmv,
                      start=(c == 0), stop=(c == NKC - 1))

              def a_step(st, d):
                  h = st.h
                  c0, c1 = 2 * d, 2 * d + 1
                  if d % 4 == 0:
                      mw = min(2048, NKC * GH - d * 512)
                      st.mslab = mpool.tile([128, mw], fp16, tag="madj")
                      nc.sync.dma_start(
                          st.mslab[:], MADJ[h][:, d * 512:d * 512 + mw])
                  zp = st.zpools[d % len(st.zpools)]
                  z = zp.tile([128, 512], f32,
                              tag=("hp" if zp is psB else "zr"))
                  f12h = (f12s[:, h * 512:(h + 1) * 512]
                          .rearrange("p (two g) -> p two g", two=2))
                  for j, c in ((0, c0), (1, c1)):
                      s12v = (s12s[:, c * 256:(c + 1) * 256]
                              .rearrange("p (two m) -> p two m", two=2))
                      nc.tensor.matmul(
                          z[:, j * GH:(j + 1) * GH], s12v, f12h,
                          perf_mode=PM.DoubleRow,
                          start=(j == 0), stop=(j == 1),
                          skip_group_check=True)
                  zl = zlpool.tile([128, 512], fp16, tag="zl")
                  st.acc += st.act_frac
                  if st.acc >= 1.0:
                      st.acc -= 1.0
                      nc.scalar.activation(zl[:], z[:], AF.Prelu,
                                           bias=zb128[:, 0:1], alpha=0.2)
                  else:
                      t02 = tpool2.tile([128, 512], fp16, tag="t02")
                      nc.vector.tensor_scalar(t02[:], z[:], 0.2, None,
                                              op0=AL.mult)
                      nc.vector.tensor_tensor(zl[:], t02[:], z[:], op=AL.max)
                  pm = pmpool.tile([128, 512], fp16, tag="pm")
                  nc.scalar.activation(pm[:], zl[:], AF.Exp,
                                       bias=zb128[:, 0:1])
                  nc.vector.tensor_tensor(
                      pm[:], pm[:],
                      st.mslab[:, (d % 4) * 512:(d % 4) * 512 + 512],
                      op=AL.mult)
                  st.pms.append(pm)
                  st.pend.append((c0, pm[:, 0:GH]))
                  st.pend.append((c1, pm[:, GH:512]))
                  while len(st.pend) > 2 * SLAG:
                      emit_sum(st, *st.pend.pop(0))

              def a_finish(st):
                  for args in st.pend:
                      emit_sum(st, *args)
                  st.pend = []
                  rS = rspool.tile([N, GH], fp8, tag="rs")
                  with nc.allow_low_precision(reason="32/s fp8 validated"):
                      nc.vector.reciprocal(rS[:], st.sP[:])
                  st.rS = rS

              def b_init(st, tail=False):
                  st.hps = [psB.tile([128, 512], f32, tag="hp",
                                     name=f"hp{st.h}_{q}") for q in range(4)]
                  st.ati = -1
                  st.att = None
                  st.avs = {}
                  st.w1vs = {}
                  st.bpend = []
                  st.tail = tail

              def emit_mlp(st, d):
                  attv = st.avs[d].rearrange("p (two g) -> p two g", two=2)
                  w1v = st.w1vs[d].rearrange("p (two oc) -> p two oc", two=2)
                  for oc in range(8):
                      nc.tensor.matmul(
                          st.hps[oc // 2][:, (oc % 2) * GH:(oc % 2 + 1) * GH],
                          w1v[:, :, oc * 128:(oc + 1) * 128], attv,
                          start=(d == 0 and oc % 2 == 0),
                          stop=(d == ND - 1),
                          perf_mode=PM.DoubleRow, skip_group_check=True)

              def b_step(st, d):
                  c0, c1 = 2 * d, 2 * d + 1
                  if st.att is None or d >= abnd[st.ati][1]:
                      st.ati += 1
                      st.att = apool.tile(
                          [128, (abnd[st.ati][1] - abnd[st.ati][0]) * 512],
                          fp8, tag="att")
                  a0 = abnd[st.ati][0]
                  if d % 2 == 0:
                      w1t = wpool.tile([128, 4096], fp8, tag="w1")
                      weng = (getattr(nc, W1_TAIL_ENGS[(d // 2) % len(W1_TAIL_ENGS)])
                              if st.tail else nc.gpsimd)
                      weng.dma_start(w1t[:], W1[d // 2])
                      st.w1vs[d] = w1t[:, 0:2048]
                      if d + 1 < ND:
                          st.w1vs[d + 1] = w1t[:, 2048:4096]
                  rf = psZR.tile([128, 512], f32, tag="zr")
                  nc.tensor.matmul(
                      rf[:, 0:GH], s12s[:, c0 * 256:c0 * 256 + 128],
                      st.rS[:], start=True, stop=False, skip_group_check=True)
                  nc.tensor.matmul(
                      rf[:, GH:512], s12s[:, c1 * 256:c1 * 256 + 128],
                      st.rS[:], start=False, stop=True, skip_group_check=True)
                  av = st.att[:, (d - a0) * 512:(d - a0 + 1) * 512]
                  nc.vector.tensor_tensor(av, st.pms[d][:], rf[:],
                                          op=AL.mult)
                  st.avs[d] = av
                  st.bpend.append(d)
                  while len(st.bpend) > BLAG:
                      emit_mlp(st, st.bpend.pop(0))

              def b_finish(st):
                  h = st.h
                  for d in st.bpend:
                      emit_mlp(st, d)
                  st.bpend = []
                  h1 = hpool.tile([128, 8 * GH], fp16, tag="h1")
                  for oc in range(8):
                      nc.scalar.activation(
                          h1[:, oc * GH:(oc + 1) * GH],
                          st.hps[oc // 2][:, (oc % 2) * GH:(oc % 2 + 1) * GH],
                          AF.Relu, bias=b1s[:, oc:oc + 1], scale=1.0)
                  h2t = psZR.tile([128, 512], f32, tag="zr")
                  h2 = h2t[0:1, :]
                  for oc in range(8):
                      for cls in range(2):
                          nc.tensor.matmul(
                              h2[0:1, cls * GH:(cls + 1) * GH],
                              w2s[:, 2 * oc + cls:2 * oc + cls + 1],
                              h1[:, oc * GH:(oc + 1) * GH],
                              start=(oc == 0 and cls == 0), stop=(oc == 7),
                              skip_group_check=True)
                  nc.scalar.activation(ca[:, h * GH:(h + 1) * GH],
                                       h2[0:1, 0:GH], AF.Identity,
                                       bias=b2s[0:1, 0:1], scale=1.0 / HSCALE)
                  nc.scalar.activation(cb[:, h * GH:(h + 1) * GH],
                                       h2[0:1, GH:2 * GH], AF.Identity,
                                       bias=b2s[0:1, 1:2], scale=1.0 / HSCALE)

              # ---- pipeline: A(0) | B(0)+A(1) | B(1) ----
              st0 = a_init(0, AF_PHASE1, [psZR, psB])
              for d in range(ND):
                  a_step(st0, d)
              a_finish(st0)

              st1 = a_init(1, AF_PHASE2, [psZR])
              b_init(st0)
              for d in range(ND):
                  b_step(st0, d)
                  a_step(st1, d)
              b_finish(st0)
              a_finish(st1)

              b_init(st1, tail=True)
              for d in range(ND):
                  b_step(st1, d)
              b_finish(st1)

              # ---- log_softmax tail ----
              nc.vector.tensor_tensor(ta[:], ca[:], cb[:], op=AL.max)
              nc.vector.tensor_tensor(ca[:], ca[:], ta[:], op=AL.subtract)
              nc.vector.tensor_tensor(cb[:], cb[:], ta[:], op=AL.subtract)
              nc.scalar.activation(ta[:], ca[:], AF.Exp, bias=zb1[:, 0:1])
              nc.scalar.activation(tb[:], cb[:], AF.Exp, bias=zb1[:, 0:1])
              nc.vector.tensor_tensor(ta[:], ta[:], tb[:], op=AL.add)
              nc.scalar.activation(ta[:], ta[:], AF.Ln, bias=zb1[:, 0:1])
              nc.vector.tensor_tensor(ca[:], ca[:], ta[:], op=AL.subtract)
              nc.vector.tensor_tensor(cb[:], cb[:], ta[:], op=AL.subtract)
              nc.sync.dma_start(OUT[0:1, :], ca[:])
              nc.sync.dma_start(OUT[1:2, :], cb[:])

            if reps == 1:
                emit_body()
            else:
                with tc.For_i(0, reps, 1) as _i:
                    emit_body()

    return nc


def _sel_arrays():
    k = np.arange(KPAD)
    ik = k // N
    jk = k % N
    valid = k < NN
    s12 = np.zeros((N, NKC * 256), F8NP)
    kk = k[valid]
    s12[ik[valid], (kk // 128) * 256 + kk % 128] = 1.0
    s12[jk, (k // 128) * 256 + 128 + k % 128] = 1.0
    rsel = np.zeros((128, NKC * N), np.float16)
    rsel[kk % 128, (kk // 128) * N + ik[valid]] = 1.0 / ATT_SCALE
    return s12, rsel


def _prep_inputs(x, adj, W_att, a1, a2, W1, b1, W2, b2):
    x = np.asarray(x, np.float32)
    adj = np.asarray(adj, np.float32)
    v12 = (np.asarray(W_att, np.float64)
           @ np.stack([np.asarray(a1, np.float64),
                       np.asarray(a2, np.float64)], 1))
    F = (x.reshape(B * N, N).astype(np.float64) @ v12).astype(np.float32)
    F = F.reshape(NCORES, G, N, 2)
    f12 = np.empty((NCORES, N, 2, 2, GH), np.float32)
    for hh in range(2):
        gs = slice(hh * GH, (hh + 1) * GH)
        f12[:, :, hh, 0, :] = F[:, gs, :, 0].transpose(0, 2, 1)
        f12[:, :, hh, 1, :] = F[:, gs, :, 1].transpose(0, 2, 1)
    f12 = np.clip(f12, -240, 240).astype(F8NP).reshape(NCORES, N, 1024)

    m01 = (adj > 0).astype(np.float16).reshape(NCORES, 2, GH, NN)
    mpad = np.zeros((NCORES, 2, GH, KPAD), np.float16)
    mpad[..., :NN] = m01
    madjt = np.ascontiguousarray(
        mpad.reshape(NCORES, 2, GH, NKC, 128).transpose(0, 1, 4, 3, 2)
        .reshape(NCORES, 2, 128, NKC * GH))

    W1p = np.zeros((KPAD, 1024), np.float32)
    W1p[:NN] = np.asarray(W1, np.float32) * W1_SCALE
    w1q = np.clip(W1p, -240, 240).astype(F8NP)
    w1q = np.ascontiguousarray(
        w1q.reshape(ND, 2, 128, 1024).transpose(0, 2, 1, 3)
        .reshape(ND, 128, 2048))
    w1p2 = np.zeros((54, 128, 2048), F8NP)
    w1p2[:ND] = w1q
    w1q = np.ascontiguousarray(
        w1p2.reshape(27, 2, 128, 2048).transpose(0, 2, 1, 3)
        .reshape(27, 128, 4096))

    w2r = np.ascontiguousarray(
        np.asarray(W2, np.float32).reshape(8, 128, 2)
        .transpose(1, 0, 2).reshape(128, 16)).astype(np.float16)
    b1r = np.ascontiguousarray(
        (np.asarray(b1, np.float32) * HSCALE).reshape(8, 128).T)
    b2c = np.ascontiguousarray(np.asarray(b2, np.float32).reshape(1, 2))

    s12, rsel = _sel_arrays()
    consts = dict(w1q=w1q, s12sel=s12, rsel=rsel,
                  w2r=w2r, b1r=b1r, b2c=b2c)
    return [dict(consts, f12=f12[c], madjt=madjt[c])
            for c in range(NCORES)]


TRACE = False
LAST_RESULTS = None


def kernel(x, adj, W_att, a1, a2, W1, b1, W2, b2):
    global LAST_RESULTS
    in_maps = _prep_inputs(x, adj, W_att, a1, a2, W1, b1, W2, b2)
    nc = build_nc()
    nc.compile()
    bres = run_bass_kernel_spmd(nc, in_maps, list(range(NCORES)), trace=TRACE)
    LAST_RESULTS = bres
    res = bres.results
    out = np.empty((B, 2), np.float32)
    for c in range(NCORES):
        out[c * G:(c + 1) * G] = np.asarray(res[c]["out"]).T
    return out



# revision 48
# speedup vs baseline: 1.1506x; 1.0009x over previous
"""Trainium2 Bass kernel for nn_GAT_22462678958399 (dense-GAT + MLP head).

Data-parallel over the 4096-graph batch across 8 NeuronCores (512/core).
The attention pipeline runs entirely in the MLP's transposed [k, g] layout
(k = i*116 + j padded to 106 chunks of 128), so the baseline's [g, k]
elementwise passes and 128x128 PE transposes disappear:

  z_c   = S1_c @ f1T + S2_c @ f2T    one dual-fp8 DoubleRow matmul per
                                     chunk (packed 0/1 selectors; F = x @
                                     W_att @ [a1,a2] is folded on host)
  zl    = lrelu(z, 0.2)              ACT Prelu, or DVE 2-pass (phase mix)
  p     = Exp(zl)                    ACT, fp16
  pm    = p * adjT(0/1)              DVE (fp16 x fp16, fast 2x/4x mode)
  s    += (R_c/32) @ pm              PE, f32 psum accumulation
  rS    = 1/s (= 32/s)               DVE reciprocal -> fp8
  rf_c  = S1_c @ rS                  PE (fp8)
  att_c = pm * rf -> fp8e4 (att*32)  DVE
  hp   += W1_c^T @ att (DoubleRow)   PE, dual-fp8 (W1 host-scaled x16)
  h1    = Relu(hp + 512*b1) fp16; h2 = w2^T @ h1; log_softmax tail.

Engine placement is HW-calibrated (see micro.py benches): gpsimd/Pool is
~3.7x slower than DVE for elementwise and cannot touch PSUM at all (BIR
verifier), so Pool only issues DMAs; DVE carries mask+att+a share of the
prelu; ACT carries Exp plus most of the prelu.  DMA queues share one
fabric (~455 GB/s single queue, ~710 aggregate), so W1 streams on the
Pool queue mid-kernel and rotates across SP/ACT/Pool in the tail; the
adjacency mask and constants stream on SP (constants chunked so the first
z matmuls start early).  The three emission phases A(0) | B(0)+A(1) |
B(1) pipeline across engines; 8 PSUM banks split psZR(3) z/rf ring +
psB(4) (extra A(0) z-depth in phase 1, MLP accumulators after) + psA(1)
softmax sums.

Quantization (validated vs f64 in CoreSim and on HW): output absmax err
~4.6e-3 on a 0.73-absmax output (gate 1.45e-2).
"""

import numpy as np
import ml_dtypes

import concourse.bass as bass
import concourse.bacc as bacc
import concourse.mybir as mybir
import concourse.tile as tile
from concourse.bass_utils import run_bass_kernel_spmd

F8NP = ml_dtypes.float8_e4m3  # IEEE e4m3 (max 240) == TRN fp8e4

N = 116
NN = N * N
NKC = 106
KPAD = NKC * 128
ND = NKC // 2
B = 4096
NCORES = 8
G = 512
GH = 256
ATT_SCALE = 32.0
W1_SCALE = 16.0
HSCALE = ATT_SCALE * W1_SCALE
ASPLIT = (18, 18, 17)
SLAG = 10
BLAG = 8
# Prelu per a-step: ACT 1-pass (Prelu) for a fraction of steps; else DVE
# 2-pass (t=0.2z ts, max(t,z) tt — one PSUM read each; gpsimd can't touch
# PSUM so both passes sit on DVE).
# HW-calibrated (micro.py): ACT act 490ns, DVE fp16 SBUF tt 249ns, DVE
# PSUM ts/tt ~290-330ns, Pool tt 915ns (useless for elementwise), PE
# [*,256] matmul ~73-80ns.  Phase 2+3 are PE-bound; phase 1 is ACT/DVE.
AF_PHASE1 = 0.344
AF_PHASE2 = 1.0
# w1 DMA queue per load in the tail B pass (no A work to compete with)
W1_TAIL_ENGS = ("sync", "scalar", "gpsimd")

f32 = mybir.dt.float32
fp16 = mybir.dt.float16
fp8 = mybir.dt.float8e4

AL = mybir.AluOpType
AF = mybir.ActivationFunctionType
PM = mybir.MatmulPerfMode


def build_nc(reps=1):
    nc = bacc.Bacc("TRN2", target_bir_lowering=False, debug=False)

    F12 = nc.dram_tensor("f12", [N, 1024], fp8, kind="ExternalInput")
    MADJ = nc.dram_tensor("madjt", [2, 128, NKC * GH], fp16, kind="ExternalInput")
    W1 = nc.dram_tensor("w1q", [27, 128, 4096], fp8, kind="ExternalInput")
    S12 = nc.dram_tensor("s12sel", [N, NKC * 256], fp8, kind="ExternalInput")
    RSEL = nc.dram_tensor("rsel", [128, NKC * N], fp16, kind="ExternalInput")
    W2 = nc.dram_tensor("w2r", [128, 16], fp16, kind="ExternalInput")
    B1 = nc.dram_tensor("b1r", [128, 8], f32, kind="ExternalInput")
    B2 = nc.dram_tensor("b2c", [1, 2], f32, kind="ExternalInput")
    OUT = nc.dram_tensor("out", [2, G], f32, kind="ExternalOutput")

    from contextlib import ExitStack
    with tile.TileContext(nc) as tc:
        with ExitStack() as es:
            pool = lambda name, bufs, space="SBUF": es.enter_context(
                tc.tile_pool(name=name, bufs=bufs, space=space))
            cpool = pool("const", 1)
            mpool = pool("madj", 3)
            tpool2 = pool("t02", 6)
            zlpool = pool("zl", 8)
            pmpool = pool("pm", ND)
            apool = pool("att", 4)
            wpool = pool("w1", 6)
            hpool = pool("h1", 1)
            tpool = pool("tail", 1)
            rspool = pool("rs", 2)
            # 8 PSUM banks total: psZR(3) + psB(4) + psA(1).  psB doubles as
            # extra z-depth for A(0) in phase 1 (the MLP accumulators are
            # idle then) and as the hp accumulators from phase 2 on — the
            # pool's WAR tracking serializes the handoff.
            psZR = pool("psZR", 3, "PSUM")
            psB = pool("psB", 4, "PSUM")
            psA = pool("psA", 1, "PSUM")

            f12s = cpool.tile_from(F12[:])
            # chunked constant loads: early chunks unblock the first z
            # matmuls ~10us sooner than one monolithic DMA would
            s12s = cpool.tile([N, NKC * 256], fp8, tag="s12s", name="s12s")
            for i in range(4):
                lo = i * 27 * 256
                hi = min(NKC, (i + 1) * 27) * 256
                nc.sync.dma_start(s12s[:, lo:hi], S12[:, lo:hi])
            rsels = cpool.tile([128, NKC * N], fp16, tag="rsels",
                               name="rsels")
            for i in range(2):
                lo = i * 53 * N
                hi = min(NKC, (i + 1) * 53) * N
                nc.sync.dma_start(rsels[:, lo:hi], RSEL[:, lo:hi])
            w2s = cpool.tile_from(W2[:])
            b1s = cpool.tile_from(B1[:])
            b2s = cpool.tile_from(B2[:])

            zb128 = cpool.tile([128, 1], f32, tag="zb128", name="zb128")
            nc.vector.memset(zb128[:], 0.0)
            zb1 = cpool.tile([1, 1], f32, tag="zb1", name="zb1")
            nc.vector.memset(zb1[:], 0.0)
            ca = tpool.tile([1, G], f32, tag="ca", name="ca")
            cb = tpool.tile([1, G], f32, tag="cb", name="cb")
            ta = tpool.tile([1, G], f32, tag="ta", name="ta")
            tb = tpool.tile([1, G], f32, tag="tb", name="tb")

            abnd = []
            o = 0
            for nt in ASPLIT:
                abnd.append((o, o + nt))
                o += nt

            def emit_body():
              class HState:
                  pass

              def a_init(h, act_frac, zpools):
                  st = HState()
                  st.h = h
                  st.sP = psA.tile([N, GH], f32, tag="aux", name=f"s{h}")
                  st.pms = []
                  st.pend = []
                  st.mslab = None
                  st.act_frac = act_frac
                  st.acc = 0.0
                  st.zpools = zpools
                  return st

              def emit_sum(st, c, pmv):
                  nc.tensor.matmul(
                      st.sP[:], rsels[:, c * N:(c + 1) * N], pmv,
                      start=(c == 0), stop=(c == NKC - 1))

              def a_step(st, d):
                  h = st.h
                  c0, c1 = 2 * d, 2 * d + 1
                  if d % 4 == 0:
                      mw = min(2048, NKC * GH - d * 512)
                      st.mslab = mpool.tile([128, mw], fp16, tag="madj")
                      nc.sync.dma_start(
                          st.mslab[:], MADJ[h][:, d * 512:d * 512 + mw])
                  zp = st.zpools[d % len(st.zpools)]
                  z = zp.tile([128, 512], f32,
                              tag=("hp" if zp is psB else "zr"))
                  f12h = (f12s[:, h * 512:(h + 1) * 512]
                          .rearrange("p (two g) -> p two g", two=2))
                  for j, c in ((0, c0), (1, c1)):
                      s12v = (s12s[:, c * 256:(c + 1) * 256]
                              .rearrange("p (two m) -> p two m", two=2))
                      nc.tensor.matmul(
                          z[:, j * GH:(j + 1) * GH], s12v, f12h,
                          perf_mode=PM.DoubleRow,
                          start=(j == 0), stop=(j == 1),
                          skip_group_check=True)
                  zl = zlpool.tile([128, 512], fp16, tag="zl")
                  st.acc += st.act_frac
                  if st.acc >= 1.0:
                      st.acc -= 1.0
                      nc.scalar.activation(zl[:], z[:], AF.Prelu,
                                           bias=zb128[:, 0:1], alpha=0.2)
                  else:
                      t02 = tpool2.tile([128, 512], fp16, tag="t02")
                      nc.vector.tensor_scalar(t02[:], z[:], 0.2, None,
                                              op0=AL.mult)
                      nc.vector.tensor_tensor(zl[:], t02[:], z[:], op=AL.max)
                  pm = pmpool.tile([128, 512], fp16, tag="pm")
                  nc.scalar.activation(pm[:], zl[:], AF.Exp,
                                       bias=zb128[:, 0:1])
                  nc.vector.tensor_tensor(
                      pm[:], pm[:],
                      st.mslab[:, (d % 4) * 512:(d % 4) * 512 + 512],
                      op=AL.mult)
                  st.pms.append(pm)
                  st.pend.append((c0, pm[:, 0:GH]))
                  st.pend.append((c1, pm[:, GH:512]))
                  while len(st.pend) > 2 * SLAG:
                      emit_sum(st, *st.pend.pop(0))

              def a_finish(st):
                  for args in st.pend:
                      emit_sum(st, *args)
                  st.pend = []
                  rS = rspool.tile([N, GH], fp8, tag="rs")
                  with nc.allow_low_precision(reason="32/s fp8 validated"):
                      nc.vector.reciprocal(rS[:], st.sP[:])
                  st.rS = rS

              def b_init(st, tail=False):
                  st.hps = [psB.tile([128, 512], f32, tag="hp",
                                     name=f"hp{st.h}_{q}") for q in range(4)]
                  st.ati = -1
                  st.att = None
                  st.avs = {}
                  st.w1vs = {}
                  st.bpend = []
                  st.tail = tail

              def emit_mlp(st, d):
                  attv = st.avs[d].rearrange("p (two g) -> p two g", two=2)
                  w1v = st.w1vs[d].rearrange("p (two oc) -> p two oc", two=2)
                  for oc in range(8):
                      nc.tensor.matmul(
                          st.hps[oc // 2][:, (oc % 2) * GH:(oc % 2 + 1) * GH],
                          w1v[:, :, oc * 128:(oc + 1) * 128], attv,
                          start=(d == 0 and oc % 2 == 0),
                          stop=(d == ND - 1),
                          perf_mode=PM.DoubleRow, skip_group_check=True)

              def b_step(st, d):
                  c0, c1 = 2 * d, 2 * d + 1
                  if st.att is None or d >= abnd[st.ati][1]:
                      st.ati += 1
                      st.att = apool.tile(
                          [128, (abnd[st.ati][1] - abnd[st.ati][0]) * 512],
                          fp8, tag="att")
                  a0 = abnd[st.ati][0]
                  if d % 2 == 0:
                      w1t = wpool.tile([128, 4096], fp8, tag="w1")
                      weng = (getattr(nc, W1_TAIL_ENGS[(d // 2) % len(W1_TAIL_ENGS)])
                              if st.tail else nc.gpsimd)
                      weng.dma_start(w1t[:], W1[d // 2])
                      st.w1vs[d] = w1t[:, 0:2048]
                      if d + 1 < ND:
                          st.w1vs[d + 1] = w1t[:, 2048:4096]
                  rf = psZR.tile([128, 512], f32, tag="zr")
                  nc.tensor.matmul(
                      rf[:, 0:GH], s12s[:, c0 * 256:c0 * 256 + 128],
                      st.rS[:], start=True, stop=False, skip_group_check=True)
                  nc.tensor.matmul(
                      rf[:, GH:512], s12s[:, c1 * 256:c1 * 256 + 128],
                      st.rS[:], start=False, stop=True, skip_group_check=True)
                  av = st.att[:, (d - a0) * 512:(d - a0 + 1) * 512]
                  nc.vector.tensor_tensor(av, st.pms[d][:], rf[:],
                                          op=AL.mult)
                  st.avs[d] = av
                  st.bpend.append(d)
                  while len(st.bpend) > BLAG:
                      emit_mlp(st, st.bpend.pop(0))

              def b_finish(st):
                  h = st.h
                  for d in st.bpend:
                      emit_mlp(st, d)
                  st.bpend = []
                  h1 = hpool.tile([128, 8 * GH], fp16, tag="h1")
                  for oc in range(8):
                      nc.scalar.activation(
                          h1[:, oc * GH:(oc + 1) * GH],
                          st.hps[oc // 2][:, (oc % 2) * GH:(oc % 2 + 1) * GH],
                          AF.Relu, bias=b1s[:, oc:oc + 1], scale=1.0)
                  h2t = psZR.tile([128, 512], f32, tag="zr")
                  h2 = h2t[0:1, :]
                  for oc in range(8):
                      for cls in range(2):
                          nc.tensor.matmul(
                              h2[0:1, cls * GH:(cls + 1) * GH],
                              w2s[:, 2 * oc + cls:2 * oc + cls + 1],
                              h1[:, oc * GH:(oc + 1) * GH],
                              start=(oc == 0 and cls == 0), stop=(oc == 7),
                              skip_group_check=True)
                  nc.scalar.activation(ca[:, h * GH:(h + 1) * GH],
                                       h2[0:1, 0:GH], AF.Identity,
                                       bias=b2s[0:1, 0:1], scale=1.0 / HSCALE)
                  nc.scalar.activation(cb[:, h * GH:(h + 1) * GH],
                                       h2[0:1, GH:2 * GH], AF.Identity,
                                       bias=b2s[0:1, 1:2], scale=1.0 / HSCALE)

              # ---- pipeline: A(0) | B(0)+A(1) | B(1) ----
              st0 = a_init(0, AF_PHASE1, [psZR, psB])
              for d in range(ND):
                  a_step(st0, d)
              a_finish(st0)

              st1 = a_init(1, AF_PHASE2, [psZR])
              b_init(st0)
              for d in range(ND):
                  b_step(st0, d)
                  a_step(st1, d)
              b_finish(st0)
              a_finish(st1)

              b_init(st1, tail=True)
              for d in range(ND):
                  b_step(st1, d)
              b_finish(st1)

              # ---- log_softmax tail ----
              nc.vector.tensor_tensor(ta[:], ca[:], cb[:], op=AL.max)
              nc.vector.tensor_tensor(ca[:], ca[:], ta[:], op=AL.subtract)
              nc.vector.tensor_tensor(cb[:], cb[:], ta[:], op=AL.subtract)
              nc.scalar.activation(ta[:], ca[:], AF.Exp, bias=zb1[:, 0:1])
              nc.scalar.activation(tb[:], cb[:], AF.Exp, bias=zb1[:, 0:1])
              nc.vector.tensor_tensor(ta[:], ta[:], tb[:], op=AL.add)
              nc.scalar.activation(ta[:], ta[:], AF.Ln, bias=zb1[:, 0:1])
              nc.vector.tensor_tensor(ca[:], ca[:], ta[:], op=AL.subtract)
              nc.vector.tensor_tensor(cb[:], cb[:], ta[:], op=AL.subtract)
              nc.sync.dma_start(OUT[0:1, :], ca[:])
              nc.sync.dma_start(OUT[1:2, :], cb[:])

            if reps == 1:
                emit_body()
            else:
                with tc.For_i(0, reps, 1) as _i:
                    emit_body()

    return nc


def _sel_arrays():
    k = np.arange(KPAD)
    ik = k // N
    jk = k % N
    valid = k < NN
    s12 = np.zeros((N, NKC * 256), F8NP)
    kk = k[valid]
    s12[ik[valid], (kk // 128) * 256 + kk % 128] = 1.0
    s12[jk, (k // 128) * 256 + 128 + k % 128] = 1.0
    rsel = np.zeros((128, NKC * N), np.float16)
    rsel[kk % 128, (kk // 128) * N + ik[valid]] = 1.0 / ATT_SCALE
    return s12, rsel


def _prep_inputs(x, adj, W_att, a1, a2, W1, b1, W2, b2):
    x = np.asarray(x, np.float32)
    adj = np.asarray(adj, np.float32)
    v12 = (np.asarray(W_att, np.float64)
           @ np.stack([np.asarray(a1, np.float64),
                       np.asarray(a2, np.float64)], 1))
    F = (x.reshape(B * N, N).astype(np.float64) @ v12).astype(np.float32)
    F = F.reshape(NCORES, G, N, 2)
    f12 = np.empty((NCORES, N, 2, 2, GH), np.float32)
    for hh in range(2):
        gs = slice(hh * GH, (hh + 1) * GH)
        f12[:, :, hh, 0, :] = F[:, gs, :, 0].transpose(0, 2, 1)
        f12[:, :, hh, 1, :] = F[:, gs, :, 1].transpose(0, 2, 1)
    f12 = np.clip(f12, -240, 240).astype(F8NP).reshape(NCORES, N, 1024)

    m01 = (adj > 0).astype(np.float16).reshape(NCORES, 2, GH, NN)
    mpad = np.zeros((NCORES, 2, GH, KPAD), np.float16)
    mpad[..., :NN] = m01
    madjt = np.ascontiguousarray(
        mpad.reshape(NCORES, 2, GH, NKC, 128).transpose(0, 1, 4, 3, 2)
        .reshape(NCORES, 2, 128, NKC * GH))

    W1p = np.zeros((KPAD, 1024), np.float32)
    W1p[:NN] = np.asarray(W1, np.float32) * W1_SCALE
    w1q = np.clip(W1p, -240, 240).astype(F8NP)
    w1q = np.ascontiguousarray(
        w1q.reshape(ND, 2, 128, 1024).transpose(0, 2, 1, 3)
        .reshape(ND, 128, 2048))
    w1p2 = np.zeros((54, 128, 2048), F8NP)
    w1p2[:ND] = w1q
    w1q = np.ascontiguousarray(
        w1p2.reshape(27, 2, 128, 2048).transpose(0, 2, 1, 3)
        .reshape(27, 128, 4096))

    w2r = np.ascontiguousarray(
        np.asarray(W2, np.float32).reshape(8, 128, 2)
        .transpose(1, 0, 2).reshape(128, 16)).astype(np.float16)
    b1r = np.ascontiguousarray(
        (np.asarray(b1, np.float32) * HSCALE).reshape(8, 128).T)
    b2c = np.ascontiguousarray(np.asarray(b2, np.float32).reshape(1, 2))

    s12, rsel = _sel_arrays()
    consts = dict(w1q=w1q, s12sel=s12, rsel=rsel,
                  w2r=w2r, b1r=b1r, b2c=b2c)
    return [dict(consts, f12=f12[c], madjt=madjt[c])
            for c in range(NCORES)]


TRACE = False
LAST_RESULTS = None


def kernel(x, adj, W_att, a1, a2, W1, b1, W2, b2):
    global LAST_RESULTS
    in_maps = _prep_inputs(x, adj, W_att, a1, a2, W1, b1, W2, b2)
    nc = build_nc()
    nc.compile()
    bres = run_bass_kernel_spmd(nc, in_maps, list(range(NCORES)), trace=TRACE)
    LAST_RESULTS = bres
    res = bres.results
    out = np.empty((B, 2), np.float32)
    for c in range(NCORES):
        out[c * G:(c + 1) * G] = np.asarray(res[c]["out"]).T
    return out

